# revision 15
# baseline (speedup 1.0000x reference)
"""Affinity-propagation (CSPN-3D) Trainium2 kernel.

Problem: guidance [24,256,256,32] f32, blur [1,256,256,32] f32.
3 iterations of (x-plane, y-plane, z-plane) 8-neighbor gated propagation:

out(q) = r(q) + c1(q) * [ sum_k G_k(q+d_k) * r(q+d_k) - S(q) * r(q) ]
  A(q) = sum_k |G_k(q+d_k)|,  S(q) = sum_k G_k(q+d_k),  c1 = 1/max(A,eps)
(equivalent to the reference's  (1-S/A)*r + (1/A)*sum_k G_k(q+d)*r(q+d))

Sharding: 8 cores, X sharded 32 rows/core with ghost margin 5,
communication free. Step 1 (the 6th X-crossing step) uses host-staggered
blur tiles + an unbaked gate-stream variant so it consumes no margin.

Layout (per core): partitions p = yb*42 + xl (3 y-thirds x 42 x-rows = 126),
free = (yc 88 = 86+2 overlap, zc 34 = 32+2 zero pad) -> FD 2992.
Gate fields are host-pre-shifted by their full neighbor offset d_k (plus the
inverse +-1 partition shift), so all device math is partition-aligned
elementwise; a PE shift-matmul accumulates the 9 slot products (8 neighbor
terms + the -S*r term) into PSUM in f32, routing the +-1 x-shift groups back
into place.
"""

import numpy as np
import ml_dtypes

BF = ml_dtypes.bfloat16

X = Y = 256
Z = 32
NCORES = 8
W = X // NCORES          # 32 interior rows per core
M = 5                    # ghost margin
S = W + 2 * M            # 42 slab rows
NYB = 3                  # y thirds
YT = 86                  # y third width
YC = YT + 2              # y cols incl 2 overlap
ZC = Z + 2               # z cols incl 2 pads
FD = YC * ZC             # 2992
P = NYB * S              # 126 partitions
NCHUNK = 4
CF = FD // NCHUNK        # 748
NHALF = 2
HF = FD // NHALF         # 1496
GUARD = 36
SLOTF = GUARD + FD + GUARD  # 3064, even
PROP_TIME = 3

# k -> (dH, dW) neighbor offsets, matching reference PADS
DLIST = [(1, 1), (1, 0), (1, -1), (0, 1), (0, -1), (-1, 1), (-1, 0), (-1, -1)]
# slot order: groups by da in {-1,0,+1}, db in {-1,0,+1} (center group 2 slots)
SLOT_DADB = [(-1, -1), (-1, 0), (-1, 1), (0, -1), (0, 1), (1, -1), (1, 0), (1, 1)]
GROUP_SLOTS = [(0, 3), (3, 5), (5, 8)]  # slot ranges per group (da=-1,0,+1)

AXES = ["x", "y", "z"]


def _axis_slots(axis):
    """Return list of 8 (channel, dx, dy, dz) in slot order for this axis."""
    base = {"x": 0, "y": 8, "z": 16}[axis]
    out = []
    for (da, db) in SLOT_DADB:
        dH, dW = da, db
        k = DLIST.index((dH, dW))
        if axis == "x":
            d = (dH, dW, 0)
        elif axis == "y":
            d = (dH, 0, dW)
        else:
            d = (0, dH, dW)
        out.append((base + k,) + d)
    return out


def _shift_full(f, dx, dy, dz):
    """Zero-padded shift: out[x,y,z] = f[x+dx, y+dy, z+dz]."""
    o = np.zeros_like(f)
    tx0, tx1 = max(0, -dx), min(X, X - dx)
    ty0, ty1 = max(0, -dy), min(Y, Y - dy)
    tz0, tz1 = max(0, -dz), min(Z, Z - dz)
    o[tx0:tx1, ty0:ty1, tz0:tz1] = f[tx0 + dx:tx1 + dx, ty0 + dy:ty1 + dy,
                                     tz0 + dz:tz1 + dz]
    return o


def _slab_L(f, x0):
    """Full field [X,Y,Z] -> core slab in L layout [P, YC, ZC] (f32)."""
    pf = np.zeros((S, Y + 4, ZC), dtype=np.float32)
    r0, r1 = x0 - M, x0 - M + S
    c0, c1 = max(0, r0), min(X, r1)
    pf[c0 - r0:c1 - r0, 1:Y + 1, 1:Z + 1] = f[c0:c1]
    return np.concatenate([pf[:, i * YT:i * YT + YC, :] for i in range(NYB)], axis=0)


_COMPILED = None
_LAST_RESULTS = None


def _build_program():
    import concourse.bacc as bacc
    import concourse.mybir as mybir
    import concourse.tile as tile

    f32 = mybir.dt.float32
    bf16 = mybir.dt.bfloat16
    MULT = mybir.AluOpType.mult
    ADD = mybir.AluOpType.add

    nc = bacc.Bacc("TRN2", target_bir_lowering=False, debug=False,
                   num_devices=NCORES)

    # ---- DRAM I/O ----
    gs = {a: nc.dram_tensor(f"gs_{a}", [NCHUNK, P, 8, CF], bf16,
                            kind="ExternalInput").ap() for a in AXES}
    gu = {a: nc.dram_tensor(f"gu_{a}", [NCHUNK, P, 8, CF], bf16,
                            kind="ExternalInput").ap() for a in AXES}
    r0_in = nc.dram_tensor("r0", [P, YC, ZC], f32, kind="ExternalInput").ap()
    r0stag = nc.dram_tensor("r0stag", [NCHUNK, P, 3, 3, CF], bf16,
                            kind="ExternalInput").ap()
    shm = nc.dram_tensor("shm", [128, 3, 128], bf16, kind="ExternalInput").ap()
    rout = nc.dram_tensor("rout", [P, YC, ZC], f32, kind="ExternalOutput").ap()

    with tile.TileContext(nc) as tc:
        with tc.tile_pool(name="stat", bufs=1) as st, \
             tc.tile_pool(name="work", bufs=1) as wk, \
             tc.tile_pool(name="fin", bufs=2) as fin, \
             tc.tile_pool(name="io", bufs=2) as io, \
             tc.tile_pool(name="psum", bufs=2, space="PSUM") as pp:

            # ---- static tiles ----
            t_r = st.tile([P, YC, ZC], f32, tag="r", name="t_r")
            t_r3 = st.tile([P, 3, SLOTF], bf16, tag="r3", name="t_r3")
            t_c1b = {a: st.tile([P, FD], bf16, tag=f"c1b{a}", name=f"t_c1b{a}")
                     for a in AXES}
            t_c0 = {a: st.tile([P, FD], f32, tag=f"c0{a}", name=f"t_c0{a}")
                    for a in AXES}
            t_shm = st.tile([128, 3, 128], bf16, tag="shm", name="t_shm")
            t_g = [st.tile([P, 8, CF], bf16, tag=f"gbuf{i}", name=f"t_g{i}")
                   for i in range(2)]
            t_carry = st.tile([P, FD], f32, tag="carry", name="t_carry")
            t_p = [st.tile([P, 8, CF], bf16, tag=f"pbuf{i}", name=f"t_p{i}")
                   for i in range(2)]

            nc.sync.dma_start(out=t_shm[:], in_=shm[:])
            nc.sync.dma_start(out=t_r[:], in_=r0_in[:])
            nc.gpsimd.memset(t_r3[:], 0.0)

            # ---- gate prep: per axis, per half, per CF2 sub-slice ----
            # A = sum|G(+d)|, S = sum G(+d), c1 = 1/max(A,eps), nS = -S
            CF2 = CF // 2
            for a in AXES:
                for ci in range(NCHUNK):
                    tgio = io.tile([P, 8, CF], bf16, tag="prepg", name="tgio")
                    dmae = nc.sync if ci % 2 == 0 else nc.scalar
                    dmae.dma_start(out=tgio[:], in_=gu[a][ci])
                    for h in range(CF // CF2):
                        hsl = slice(h * CF2, (h + 1) * CF2)
                        csl = slice(ci * CF + h * CF2, ci * CF + (h + 1) * CF2)
                        tg = tgio[:, :, hsl]
                        tabs = wk.tile([P, 8, CF2], bf16, tag="prepabs",
                                       name="tabs")
                        nc.vector.tensor_scalar(
                            tabs[:].bitcast(mybir.dt.int16),
                            tg.bitcast(mybir.dt.int16), 0x7FFF, None,
                            mybir.AluOpType.bitwise_and)
                        # A tree: L1 bf16, then f32
                        pa = wk.tile([P, 4, CF2], bf16, tag="prep_pa", name="pa")
                        nc.vector.tensor_tensor(out=pa[:], in0=tabs[:, 0:8:2, :],
                                                in1=tabs[:, 1:8:2, :], op=ADD)
                        pa2 = wk.tile([P, 2, CF2], f32, tag="prep_pa2",
                                      name="pa2")
                        nc.vector.tensor_tensor(out=pa2[:], in0=pa[:, 0:4:2, :],
                                                in1=pa[:, 1:4:2, :], op=ADD)
                        tA = wk.tile([P, CF2], f32, tag="prep_A", name="tA")
                        nc.vector.tensor_tensor(out=tA[:], in0=pa2[:, 0, :],
                                                in1=pa2[:, 1, :], op=ADD)
                        # S tree: L1 on gpsimd, rest gpsimd; nS = -S in bf16
                        ps1 = wk.tile([P, 4, CF2], bf16, tag="prep_ps",
                                      name="ps1")
                        nc.gpsimd.tensor_tensor(out=ps1[:], in0=tg[:, 0:8:2, :],
                                                in1=tg[:, 1:8:2, :], op=ADD)
                        ps2 = wk.tile([P, 2, CF2], f32, tag="prep_ps2",
                                      name="ps2")
                        nc.vector.tensor_tensor(out=ps2[:], in0=ps1[:, 0:4:2, :],
                                                in1=ps1[:, 1:4:2, :], op=ADD)
                        tS = wk.tile([P, CF2], f32, tag="prep_S", name="tS")
                        nc.gpsimd.tensor_tensor(out=tS[:], in0=ps2[:, 0, :],
                                                in1=ps2[:, 1, :], op=ADD)
                        # c1 = 1/max(A, eps)
                        nc.vector.tensor_scalar_max(tA[:], tA[:], 1e-30)
                        tc1 = wk.tile([P, CF2], f32, tag="prep_c1", name="tc1")
                        nc.vector.reciprocal_approx_fast(tc1[:], tA[:])
                        nc.scalar.activation(t_c1b[a][:, csl], tc1[:],
                                             mybir.ActivationFunctionType.Copy)
                        # c0 = 1 - S*c1
                        tSc = wk.tile([P, CF2], f32, tag="prep_sc", name="tSc")
                        nc.gpsimd.tensor_tensor(out=tSc[:], in0=tS[:],
                                                in1=tc1[:], op=MULT)
                        nc.scalar.activation(t_c0[a][:, csl], tSc[:],
                                             mybir.ActivationFunctionType.Identity,
                                             bias=1.0, scale=-1.0)

            # ---- propagation steps ----
            # matmul order: center group (incl -S slot) first, then m1, p1 --
            # consecutive matmuls share the stationary shift matrix.
            MM_ORDER = [(3, 1), (4, 1),
                        (0, 0), (1, 0), (2, 0),
                        (5, 2), (6, 2), (7, 2)]
            step = 0
            for it in range(PROP_TIME):
                for a in AXES:
                    step += 1
                    first = (step == 1)
                    dbu = ZC if a == "x" else 1
                    da_free = a == "z"

                    if not first:
                        # refresh y-overlap cols of r (SBUF->SBUF DMA;
                        # partition-offset copies are illegal on compute)
                        nc.sync.dma_start(out=t_r[S:P, 0, :],
                                          in_=t_r[0:P - S, YT, :])
                        nc.sync.dma_start(out=t_r[0:P - S, YC - 1, :],
                                          in_=t_r[S:P, 1, :])
                        # r3 slot1 = bf16(r); slot0/2 = shifted by -+dbu
                        rf = t_r[:].rearrange("p a b -> p (a b)")
                        nc.scalar.activation(
                            t_r3[:, 1, GUARD:GUARD + FD], rf,
                            mybir.ActivationFunctionType.Copy)
                        nc.scalar.activation(
                            t_r3[:, 0, GUARD:GUARD + FD],
                            t_r3[:, 1, GUARD - dbu:GUARD + FD - dbu],
                            mybir.ActivationFunctionType.Copy)
                        nc.scalar.activation(
                            t_r3[:, 2, GUARD:GUARD + FD],
                            t_r3[:, 1, GUARD + dbu:GUARD + FD + dbu],
                            mybir.ActivationFunctionType.Copy)

                    rfall = t_r[:].rearrange("p a b -> p (a b)")
                    nc.gpsimd.tensor_tensor(out=t_carry[:], in0=t_c0[a][:],
                                            in1=rfall, op=MULT)
                    for c in range(NCHUNK):
                        buf = (step * NCHUNK + c) % 2
                        dmae = nc.sync if c % 2 == 0 else nc.scalar
                        src_gs = gu["x"] if first else gs[a]
                        dmae.dma_start(out=t_g[buf][:], in_=src_gs[c])
                        tg_ = t_g[buf]
                        gsl = slice(0, CF)
                        if first:
                            stag_t = wk.tile([P, 3, 3, CF], bf16,
                                             tag="stagc", name="stag_t")
                            nc.sync.dma_start(out=stag_t[:], in_=r0stag[c])
                        # products per group (stacked over slots)
                        for gi, (s0, s1) in enumerate(GROUP_SLOTS):
                            nsl = s1 - s0
                            if first:
                                if nsl == 3:
                                    in1 = stag_t[:, gi, 0:3, :]
                                else:
                                    in1 = stag_t[:, gi, 0:3:2, :]
                            else:
                                base = GUARD + c * CF
                                if da_free:
                                    base += (gi - 1) * ZC
                                if nsl == 3:
                                    in1 = t_r3[:, 0:3, base:base + CF]
                                else:
                                    in1 = t_r3[:, 0:3:2, base:base + CF]
                            eng = nc.vector
                            eng.tensor_tensor(out=t_p[buf][:, s0:s1, :],
                                              in0=tg_[:, s0:s1, gsl],
                                              in1=in1, op=MULT)
                        # PE shift-matmul accumulate all 8 slots into PSUM
                        tps = pp.tile([P, CF], f32, tag="ps", name="tps")
                        for n0 in range(0, CF, 512):
                            n1 = min(CF, n0 + 512)
                            for mi, (s, gi) in enumerate(MM_ORDER):
                                smi = 1 if (first or da_free) else gi
                                nc.tensor.matmul(
                                    tps[:, n0:n1],
                                    t_shm[0:P, smi, 0:P],
                                    t_p[buf][:, s, n0:n1],
                                    start=(mi == 0), stop=(mi == 7))
                        # out chunk = c0*r + c1b*psum (writes r in place)
                        rfc = t_r[:].rearrange("p a b -> p (a b)")
                        tmul = fin.tile([P, CF], f32, tag="tmul", name="tmul")
                        nc.vector.tensor_tensor(
                            out=tmul[:],
                            in0=t_c1b[a][:, c * CF:(c + 1) * CF],
                            in1=tps[:], op=MULT)
                        nc.gpsimd.tensor_add(
                            out=rfc[:, c * CF:(c + 1) * CF],
                            in0=t_carry[:, c * CF:(c + 1) * CF],
                            in1=tmul[:])

            nc.sync.dma_start(out=rout[:], in_=t_r[:])

    nc.compile()
    return nc


def _prep_inputs(guidance, blur):
    """Host-side swizzle: build per-core input dicts."""
    guidance = np.asarray(guidance, dtype=np.float32)
    blur = np.asarray(blur, dtype=np.float32)[0]  # [X,Y,Z]
    x0s = [c * W for c in range(NCORES)]

    in_maps = [dict() for _ in range(NCORES)]

    # shift matrices: SM[q, g, m]: g=0: m=q+1 ; g=1: m=q ; g=2: m=q-1
    sm = np.zeros((128, 3, 128), dtype=BF)
    for q in range(P):
        if q + 1 < P:
            sm[q, 0, q + 1] = 1.0
        sm[q, 1, q] = 1.0
        if q - 1 >= 0:
            sm[q, 2, q - 1] = 1.0
    for c in range(NCORES):
        in_maps[c]["shm"] = sm

    # gate stacks, pre-shifted by full neighbor offset; the +-1 partition
    # (x) shift of the product routing is also baked per slot (slab start
    # x0 - da), except in the unbaked step-1 variant of axis x.
    for a in AXES:
        slots = _axis_slots(a)
        shifted = np.empty((8, X, Y, Z), dtype=np.float32)
        for si, (ch, dx, dy, dz) in enumerate(slots):
            shifted[si] = _shift_full(guidance[ch], dx, dy, dz)
        variants = [(f"gs_{a}", True), (f"gu_{a}", False)]
        for name, baked in variants:
            for c in range(NCORES):
                L = np.empty((P, 8, YC, ZC), dtype=np.float32)
                for si in range(8):
                    da = SLOT_DADB[si][0]
                    if a == "z" or not baked:
                        da = 0
                    L[:, si] = _slab_L(shifted[si], x0s[c] - da)
                Lh = L.reshape(P, 8, FD).reshape(P, 8, NCHUNK, CF)
                in_maps[c][name] = np.ascontiguousarray(
                    Lh.transpose(2, 0, 1, 3)).astype(BF)

    # r0 + staggered step-1 triples (axis x: da in x, db in y)
    for c in range(NCORES):
        in_maps[c]["r0"] = _slab_L(blur, x0s[c])
    stag = np.empty((3, 3, X, Y, Z), dtype=np.float32)
    for gi, da in enumerate((-1, 0, 1)):
        for j, db in enumerate((-1, 0, 1)):
            stag[gi, j] = _shift_full(blur, da, db, 0)
    for c in range(NCORES):
        stc = np.empty((P, 3, 3, FD), dtype=np.float32)
        for gi in range(3):
            for j in range(3):
                stc[:, gi, j] = _slab_L(stag[gi, j], x0s[c]).reshape(P, FD)
        stc = stc.reshape(P, 3, 3, NCHUNK, CF).transpose(3, 0, 1, 2, 4)
        in_maps[c]["r0stag"] = np.ascontiguousarray(stc).astype(BF)

    return in_maps


def _unswizzle(results):
    out = np.empty((1, X, Y, Z), dtype=np.float32)
    for c in range(NCORES):
        r = results[c]["rout"]  # [P, YC, ZC]
        x0 = c * W
        for yb in range(NYB):
            ys = yb * YT
            ye = min(Y, ys + YT)
            out[0, x0:x0 + W, ys:ye, :] = \
                r[yb * S + M: yb * S + M + W, 1:1 + (ye - ys), 1:Z + 1]
    return out


def kernel(guidance, blur):
    global _COMPILED, _LAST_RESULTS
    from concourse import bass_utils
    if _COMPILED is None:
        _COMPILED = _build_program()
    nc = _COMPILED
    in_maps = _prep_inputs(guidance, blur)
    res = bass_utils.run_bass_kernel_spmd(nc, in_maps,
                                          core_ids=list(range(NCORES)))
    _LAST_RESULTS = res
    return _unswizzle(res.results)


# revision 16
# speedup vs baseline: 1.1226x; 1.1226x over previous
"""Affinity-propagation (CSPN-3D) Trainium2 kernel.

Problem: guidance [24,256,256,32] f32, blur [1,256,256,32] f32.
3 iterations of (x-plane, y-plane, z-plane) 8-neighbor gated propagation:

out(q) = r(q) + c1(q) * [ sum_k G_k(q+d_k) * r(q+d_k) - S(q) * r(q) ]
  A(q) = sum_k |G_k(q+d_k)|,  S(q) = sum_k G_k(q+d_k),  c1 = 1/max(A,eps)
(equivalent to the reference's  (1-S/A)*r + (1/A)*sum_k G_k(q+d)*r(q+d))

Sharding: 8 cores, X sharded 32 rows/core with ghost margin 5,
communication free. Step 1 (the 6th X-crossing step) uses host-staggered
blur tiles + an unbaked gate-stream variant so it consumes no margin.

Layout (per core): partitions p = yb*42 + xl (3 y-thirds x 42 x-rows = 126),
free = (yc 88 = 86+2 overlap, zc 34 = 32+2 zero pad) -> FD 2992.
Gate fields are host-pre-shifted by their full neighbor offset d_k (plus the
inverse +-1 partition shift), so all device math is partition-aligned
elementwise; a PE shift-matmul accumulates the 9 slot products (8 neighbor
terms + the -S*r term) into PSUM in f32, routing the +-1 x-shift groups back
into place.
"""

import numpy as np
import ml_dtypes

BF = ml_dtypes.bfloat16

X = Y = 256
Z = 32
NCORES = 8
W = X // NCORES          # 32 interior rows per core
M = 5                    # ghost margin
S = W + 2 * M            # 42 slab rows
NYB = 3                  # y thirds
YT = 86                  # y third width
YC = YT + 2              # y cols incl 2 overlap
ZC = Z + 2               # z cols incl 2 pads
FD = YC * ZC             # 2992
P = NYB * S              # 126 partitions
NCHUNK = 4
CF = FD // NCHUNK        # 748
NHALF = 2
HF = FD // NHALF         # 1496
GUARD = 36
SLOTF = GUARD + FD + GUARD  # 3064, even
PROP_TIME = 3

# k -> (dH, dW) neighbor offsets, matching reference PADS
DLIST = [(1, 1), (1, 0), (1, -1), (0, 1), (0, -1), (-1, 1), (-1, 0), (-1, -1)]
# slot order: groups by da in {-1,0,+1}, db in {-1,0,+1} (center group 2 slots)
SLOT_DADB = [(-1, -1), (-1, 0), (-1, 1), (0, -1), (0, 1), (1, -1), (1, 0), (1, 1)]
GROUP_SLOTS = [(0, 3), (3, 5), (5, 8)]  # slot ranges per group (da=-1,0,+1)

AXES = ["x", "y", "z"]


def _axis_slots(axis):
    """Return list of 8 (channel, dx, dy, dz) in slot order for this axis."""
    base = {"x": 0, "y": 8, "z": 16}[axis]
    out = []
    for (da, db) in SLOT_DADB:
        dH, dW = da, db
        k = DLIST.index((dH, dW))
        if axis == "x":
            d = (dH, dW, 0)
        elif axis == "y":
            d = (dH, 0, dW)
        else:
            d = (0, dH, dW)
        out.append((base + k,) + d)
    return out


def _shift_full(f, dx, dy, dz):
    """Zero-padded shift: out[x,y,z] = f[x+dx, y+dy, z+dz]."""
    o = np.zeros_like(f)
    tx0, tx1 = max(0, -dx), min(X, X - dx)
    ty0, ty1 = max(0, -dy), min(Y, Y - dy)
    tz0, tz1 = max(0, -dz), min(Z, Z - dz)
    o[tx0:tx1, ty0:ty1, tz0:tz1] = f[tx0 + dx:tx1 + dx, ty0 + dy:ty1 + dy,
                                     tz0 + dz:tz1 + dz]
    return o


def _slab_L(f, x0):
    """Full field [X,Y,Z] -> core slab in L layout [P, YC, ZC] (f32)."""
    pf = np.zeros((S, Y + 4, ZC), dtype=np.float32)
    r0, r1 = x0 - M, x0 - M + S
    c0, c1 = max(0, r0), min(X, r1)
    pf[c0 - r0:c1 - r0, 1:Y + 1, 1:Z + 1] = f[c0:c1]
    return np.concatenate([pf[:, i * YT:i * YT + YC, :] for i in range(NYB)], axis=0)


_COMPILED = None
_LAST_RESULTS = None


def _build_program():
    import concourse.bacc as bacc
    import concourse.mybir as mybir
    import concourse.tile as tile

    f32 = mybir.dt.float32
    bf16 = mybir.dt.bfloat16
    MULT = mybir.AluOpType.mult
    ADD = mybir.AluOpType.add

    nc = bacc.Bacc("TRN2", target_bir_lowering=False, debug=False,
                   num_devices=NCORES)

    # ---- DRAM I/O ----
    gs = {a: nc.dram_tensor(f"gs_{a}", [NCHUNK, P, 8, CF], bf16,
                            kind="ExternalInput").ap() for a in AXES}
    gu = {a: nc.dram_tensor(f"gu_{a}", [NCHUNK, P, 8, CF], bf16,
                            kind="ExternalInput").ap() for a in AXES}
    r0_in = nc.dram_tensor("r0", [P, YC, ZC], f32, kind="ExternalInput").ap()
    r0stag = nc.dram_tensor("r0stag", [NCHUNK, P, 3, 3, CF], bf16,
                            kind="ExternalInput").ap()
    shm = nc.dram_tensor("shm", [128, 3, 128], bf16, kind="ExternalInput").ap()
    rout = nc.dram_tensor("rout", [P, YC, ZC], f32, kind="ExternalOutput").ap()

    with tile.TileContext(nc) as tc:
        with tc.tile_pool(name="stat", bufs=1) as st, \
             tc.tile_pool(name="work", bufs=1) as wk, \
             tc.tile_pool(name="fin", bufs=2) as fin, \
             tc.tile_pool(name="io", bufs=2) as io, \
             tc.tile_pool(name="psum", bufs=2, space="PSUM") as pp:

            # ---- static tiles ----
            t_r = st.tile([P, YC, ZC], f32, tag="r", name="t_r")
            t_r3 = st.tile([P, 3, SLOTF], bf16, tag="r3", name="t_r3")
            t_c1b = {a: st.tile([P, FD], bf16, tag=f"c1b{a}", name=f"t_c1b{a}")
                     for a in AXES}
            t_c0 = {a: st.tile([P, FD], f32, tag=f"c0{a}", name=f"t_c0{a}")
                    for a in AXES}
            t_shm = st.tile([128, 3, 128], bf16, tag="shm", name="t_shm")
            t_g = [st.tile([P, 8, CF], bf16, tag=f"gbuf{i}", name=f"t_g{i}")
                   for i in range(2)]
            t_carry = st.tile([P, FD], f32, tag="carry", name="t_carry")
            t_p = [st.tile([P, 8, CF], bf16, tag=f"pbuf{i}", name=f"t_p{i}")
                   for i in range(2)]

            nc.sync.dma_start(out=t_shm[:], in_=shm[:])
            nc.sync.dma_start(out=t_r[:], in_=r0_in[:])
            nc.gpsimd.memset(t_r3[:], 0.0)

            # ---- gate prep: per axis, per half, per CF2 sub-slice ----
            # A = sum|G(+d)|, S = sum G(+d), c1 = 1/max(A,eps), nS = -S
            CF2 = CF // 2

            def prep_axis(a):
                for ci in range(NCHUNK):
                    tgio = io.tile([P, 8, CF], bf16, tag="prepg", name="tgio")
                    dmae = nc.sync if ci % 2 == 0 else nc.scalar
                    dmae.dma_start(out=tgio[:], in_=gu[a][ci])
                    for h in range(CF // CF2):
                        hsl = slice(h * CF2, (h + 1) * CF2)
                        csl = slice(ci * CF + h * CF2, ci * CF + (h + 1) * CF2)
                        tg = tgio[:, :, hsl]
                        tabs = wk.tile([P, 8, CF2], bf16, tag="prepabs",
                                       name="tabs")
                        nc.vector.tensor_scalar(
                            tabs[:].bitcast(mybir.dt.int16),
                            tg.bitcast(mybir.dt.int16), 0x7FFF, None,
                            mybir.AluOpType.bitwise_and)
                        # A tree: L1 bf16, then f32
                        pa = wk.tile([P, 4, CF2], bf16, tag="prep_pa", name="pa")
                        nc.vector.tensor_tensor(out=pa[:], in0=tabs[:, 0:8:2, :],
                                                in1=tabs[:, 1:8:2, :], op=ADD)
                        pa2 = wk.tile([P, 2, CF2], f32, tag="prep_pa2",
                                      name="pa2")
                        nc.vector.tensor_tensor(out=pa2[:], in0=pa[:, 0:4:2, :],
                                                in1=pa[:, 1:4:2, :], op=ADD)
                        tA = wk.tile([P, CF2], f32, tag="prep_A", name="tA")
                        nc.vector.tensor_tensor(out=tA[:], in0=pa2[:, 0, :],
                                                in1=pa2[:, 1, :], op=ADD)
                        # S tree: L1 on gpsimd, rest gpsimd; nS = -S in bf16
                        ps1 = wk.tile([P, 4, CF2], bf16, tag="prep_ps",
                                      name="ps1")
                        nc.gpsimd.tensor_tensor(out=ps1[:], in0=tg[:, 0:8:2, :],
                                                in1=tg[:, 1:8:2, :], op=ADD)
                        ps2 = wk.tile([P, 2, CF2], f32, tag="prep_ps2",
                                      name="ps2")
                        nc.vector.tensor_tensor(out=ps2[:], in0=ps1[:, 0:4:2, :],
                                                in1=ps1[:, 1:4:2, :], op=ADD)
                        tS = wk.tile([P, CF2], f32, tag="prep_S", name="tS")
                        nc.gpsimd.tensor_tensor(out=tS[:], in0=ps2[:, 0, :],
                                                in1=ps2[:, 1, :], op=ADD)
                        # c1 = 1/max(A, eps)
                        nc.vector.tensor_scalar_max(tA[:], tA[:], 1e-30)
                        tc1 = wk.tile([P, CF2], f32, tag="prep_c1", name="tc1")
                        nc.vector.reciprocal_approx_fast(tc1[:], tA[:])
                        nc.scalar.activation(t_c1b[a][:, csl], tc1[:],
                                             mybir.ActivationFunctionType.Copy)
                        # c0 = 1 - S*c1
                        tSc = wk.tile([P, CF2], f32, tag="prep_sc", name="tSc")
                        nc.gpsimd.tensor_tensor(out=tSc[:], in0=tS[:],
                                                in1=tc1[:], op=MULT)
                        nc.scalar.activation(t_c0[a][:, csl], tSc[:],
                                             mybir.ActivationFunctionType.Identity,
                                             bias=1.0, scale=-1.0)

            prep_axis("x")

            # ---- propagation steps ----
            # matmul order: center group (incl -S slot) first, then m1, p1 --
            # consecutive matmuls share the stationary shift matrix.
            MM_ORDER = [(3, 1), (4, 1),
                        (0, 0), (1, 0), (2, 0),
                        (5, 2), (6, 2), (7, 2)]
            step = 0
            for it in range(PROP_TIME):
                for a in AXES:
                    step += 1
                    if step == 2:
                        prep_axis("y")
                    elif step == 3:
                        prep_axis("z")
                    first = (step == 1)
                    dbu = ZC if a == "x" else 1
                    da_free = a == "z"

                    if not first:
                        # refresh y-overlap cols of r (SBUF->SBUF DMA;
                        # partition-offset copies are illegal on compute)
                        nc.sync.dma_start(out=t_r[S:P, 0, :],
                                          in_=t_r[0:P - S, YT, :])
                        nc.sync.dma_start(out=t_r[0:P - S, YC - 1, :],
                                          in_=t_r[S:P, 1, :])
                        # r3 slot1 = bf16(r); slot0/2 = shifted by -+dbu
                        rf = t_r[:].rearrange("p a b -> p (a b)")
                        nc.scalar.activation(
                            t_r3[:, 1, GUARD:GUARD + FD], rf,
                            mybir.ActivationFunctionType.Copy)
                        nc.scalar.activation(
                            t_r3[:, 0, GUARD:GUARD + FD],
                            t_r3[:, 1, GUARD - dbu:GUARD + FD - dbu],
                            mybir.ActivationFunctionType.Copy)
                        nc.scalar.activation(
                            t_r3[:, 2, GUARD:GUARD + FD],
                            t_r3[:, 1, GUARD + dbu:GUARD + FD + dbu],
                            mybir.ActivationFunctionType.Copy)

                    rfall = t_r[:].rearrange("p a b -> p (a b)")
                    nc.gpsimd.tensor_tensor(out=t_carry[:], in0=t_c0[a][:],
                                            in1=rfall, op=MULT)
                    for c in range(NCHUNK):
                        buf = (step * NCHUNK + c) % 2
                        dmae = nc.sync if c % 2 == 0 else nc.scalar
                        src_gs = gu["x"] if first else gs[a]
                        dmae.dma_start(out=t_g[buf][:], in_=src_gs[c])
                        tg_ = t_g[buf]
                        gsl = slice(0, CF)
                        if first:
                            stag_t = wk.tile([P, 3, 3, CF], bf16,
                                             tag="stagc", name="stag_t")
                            nc.sync.dma_start(out=stag_t[:], in_=r0stag[c])
                        # products per group (stacked over slots)
                        for gi, (s0, s1) in enumerate(GROUP_SLOTS):
                            nsl = s1 - s0
                            if first:
                                if nsl == 3:
                                    in1 = stag_t[:, gi, 0:3, :]
                                else:
                                    in1 = stag_t[:, gi, 0:3:2, :]
                            else:
                                base = GUARD + c * CF
                                if da_free:
                                    base += (gi - 1) * ZC
                                if nsl == 3:
                                    in1 = t_r3[:, 0:3, base:base + CF]
                                else:
                                    in1 = t_r3[:, 0:3:2, base:base + CF]
                            eng = nc.vector
                            eng.tensor_tensor(out=t_p[buf][:, s0:s1, :],
                                              in0=tg_[:, s0:s1, gsl],
                                              in1=in1, op=MULT)
                        # PE shift-matmul accumulate all 8 slots into PSUM
                        tps = pp.tile([P, CF], f32, tag="ps", name="tps")
                        for n0 in range(0, CF, 512):
                            n1 = min(CF, n0 + 512)
                            for mi, (s, gi) in enumerate(MM_ORDER):
                                smi = 1 if (first or da_free) else gi
                                nc.tensor.matmul(
                                    tps[:, n0:n1],
                                    t_shm[0:P, smi, 0:P],
                                    t_p[buf][:, s, n0:n1],
                                    start=(mi == 0), stop=(mi == 7))
                        # out chunk = c0*r + c1b*psum (writes r in place)
                        rfc = t_r[:].rearrange("p a b -> p (a b)")
                        tmul = fin.tile([P, CF], f32, tag="tmul", name="tmul")
                        nc.vector.tensor_tensor(
                            out=tmul[:],
                            in0=t_c1b[a][:, c * CF:(c + 1) * CF],
                            in1=tps[:], op=MULT)
                        nc.gpsimd.tensor_add(
                            out=rfc[:, c * CF:(c + 1) * CF],
                            in0=t_carry[:, c * CF:(c + 1) * CF],
                            in1=tmul[:])

            nc.sync.dma_start(out=rout[:], in_=t_r[:])

    nc.compile()
    return nc


def _prep_inputs(guidance, blur):
    """Host-side swizzle: build per-core input dicts."""
    guidance = np.asarray(guidance, dtype=np.float32)
    blur = np.asarray(blur, dtype=np.float32)[0]  # [X,Y,Z]
    x0s = [c * W for c in range(NCORES)]

    in_maps = [dict() for _ in range(NCORES)]

    # shift matrices: SM[q, g, m]: g=0: m=q+1 ; g=1: m=q ; g=2: m=q-1
    sm = np.zeros((128, 3, 128), dtype=BF)
    for q in range(P):
        if q + 1 < P:
            sm[q, 0, q + 1] = 1.0
        sm[q, 1, q] = 1.0
        if q - 1 >= 0:
            sm[q, 2, q - 1] = 1.0
    for c in range(NCORES):
        in_maps[c]["shm"] = sm

    # gate stacks, pre-shifted by full neighbor offset; the +-1 partition
    # (x) shift of the product routing is also baked per slot (slab start
    # x0 - da), except in the unbaked step-1 variant of axis x.
    for a in AXES:
        slots = _axis_slots(a)
        shifted = np.empty((8, X, Y, Z), dtype=np.float32)
        for si, (ch, dx, dy, dz) in enumerate(slots):
            shifted[si] = _shift_full(guidance[ch], dx, dy, dz)
        variants = [(f"gs_{a}", True), (f"gu_{a}", False)]
        for name, baked in variants:
            for c in range(NCORES):
                L = np.empty((P, 8, YC, ZC), dtype=np.float32)
                for si in range(8):
                    da = SLOT_DADB[si][0]
                    if a == "z" or not baked:
                        da = 0
                    L[:, si] = _slab_L(shifted[si], x0s[c] - da)
                Lh = L.reshape(P, 8, FD).reshape(P, 8, NCHUNK, CF)
                in_maps[c][name] = np.ascontiguousarray(
                    Lh.transpose(2, 0, 1, 3)).astype(BF)

    # r0 + staggered step-1 triples (axis x: da in x, db in y)
    for c in range(NCORES):
        in_maps[c]["r0"] = _slab_L(blur, x0s[c])
    stag = np.empty((3, 3, X, Y, Z), dtype=np.float32)
    for gi, da in enumerate((-1, 0, 1)):
        for j, db in enumerate((-1, 0, 1)):
            stag[gi, j] = _shift_full(blur, da, db, 0)
    for c in range(NCORES):
        stc = np.empty((P, 3, 3, FD), dtype=np.float32)
        for gi in range(3):
            for j in range(3):
                stc[:, gi, j] = _slab_L(stag[gi, j], x0s[c]).reshape(P, FD)
        stc = stc.reshape(P, 3, 3, NCHUNK, CF).transpose(3, 0, 1, 2, 4)
        in_maps[c]["r0stag"] = np.ascontiguousarray(stc).astype(BF)

    return in_maps


def _unswizzle(results):
    out = np.empty((1, X, Y, Z), dtype=np.float32)
    for c in range(NCORES):
        r = results[c]["rout"]  # [P, YC, ZC]
        x0 = c * W
        for yb in range(NYB):
            ys = yb * YT
            ye = min(Y, ys + YT)
            out[0, x0:x0 + W, ys:ye, :] = \
                r[yb * S + M: yb * S + M + W, 1:1 + (ye - ys), 1:Z + 1]
    return out


def kernel(guidance, blur):
    global _COMPILED, _LAST_RESULTS
    from concourse import bass_utils
    if _COMPILED is None:
        _COMPILED = _build_program()
    nc = _COMPILED
    in_maps = _prep_inputs(guidance, blur)
    res = bass_utils.run_bass_kernel_spmd(nc, in_maps,
                                          core_ids=list(range(NCORES)))
    _LAST_RESULTS = res
    return _unswizzle(res.results)
